# revision 1
# baseline (speedup 1.0000x reference)
"""GAT (4-layer, 8-head) + GraphNorm kernel for 8 TRN2 NeuronCores.

Strategy: destination-sharded message passing. Each core owns N/8 nodes and
all edges pointing at them. Per layer, a gather table of per-node rows
z = h @ (Wg @ blockdiag(T_h)) (bf16, 256B rows) is replicated to every core
via AllGather; T_h's first column equals att_src[h] so the per-edge source
attention term is z[16h] for free. Messages are fetched with dma_gather
(4 SWDGE queues, negative-int16 index trick to cover >32K rows), attention
is computed per (node, slot) on ACT/DVE, and aggregation runs on the PE as
identity-matmul PSUM accumulation over slots. GraphNorm statistics are
per-block PE matmuls against per-core one-hot graph-membership tiles,
combined with a 16KB AllReduce.
"""

import sys

import numpy as np

if "/opt/trn_rl_repo" not in sys.path:
    sys.path.insert(0, "/opt/trn_rl_repo")

# ---------------------------------------------------------------- config

N_CORES = 8
H = 8
DH = 16
HID = 128
EPS = 1e-5
NEG_SLOPE = 0.2
PAD_ES = -80.0  # es value stored in the pad table row; kills pad-slot alphas

FULL_CFG = dict(N=50000, E=800000, G=16, L=4)


def _derive(cfg):
    N = cfg["N"]
    npc = N // N_CORES                      # real nodes per core
    nblk = (npc + 127) // 128               # 128-node blocks per core
    npad = nblk * 128                       # padded nodes per core
    trows = N_CORES * npad                  # global table rows
    tbase = max(0, trows - 32767)           # gather base row (neg-idx trick)
    assert npc < npad, "need at least one pad node for the pad table row"
    pad_row = trows - 1                     # last pad node of core 7
    assert pad_row - tbase >= 0
    return dict(NPC=npc, NBLK=nblk, NPAD=npad, TROWS=trows, TBASE=tbase,
                PAD_ROW=pad_row, **cfg)


# ---------------------------------------------------------- host preprocess


def _padrow():
    import ml_dtypes
    r = np.zeros((1, HID), np.float32)
    r[0, 0::DH] = PAD_ES
    return r.astype(ml_dtypes.bfloat16)


def _preprocess(inputs, cfg):
    """All numpy. Returns per-core data + SPMD-uniform shape info."""
    d = _derive(cfg)
    N, G, L = d["N"], d["G"], d["L"]
    NPC, NBLK, NPAD, TBASE, PAD_ROW = (
        d["NPC"], d["NBLK"], d["NPAD"], d["TBASE"], d["PAD_ROW"])

    x = np.asarray(inputs["x"], np.float32)
    ei = np.asarray(inputs["edge_index"], np.int64)
    batch = np.asarray(inputs["batch"], np.int64).astype(np.int32)
    loops = np.arange(N, dtype=np.int64)
    src = np.concatenate([ei[0], loops]).astype(np.int64)
    dst = np.concatenate([ei[1], loops]).astype(np.int64)

    # ---- per-core node permutation: degree-sort (desc) within graph runs
    deg_all = np.bincount(dst, minlength=N)
    perms = []          # perms[c][new_pos] = orig local id
    for c in range(N_CORES):
        lo, hi = c * NPC, (c + 1) * NPC
        b = batch[lo:hi]
        degl = deg_all[lo:hi]
        # stable sort key: (graph id asc, degree desc)
        order = np.lexsort((-degl, b))
        perms.append(order.astype(np.int64))
    inv_perms = [np.argsort(p) for p in perms]

    # global padded row id of an original node id
    def padded_row(orig):
        c = orig // NPC
        return c * NPAD + inv_perms[c][orig - c * NPC]

    row_of = np.empty(N, np.int64)
    for c in range(N_CORES):
        row_of[c * NPC:(c + 1) * NPC] = c * NPAD + inv_perms[c]

    # ---- per-core edge slot grids
    # block max degrees, SPMD-uniform: max over cores per block index
    deg_perm = [deg_all[c * NPC:(c + 1) * NPC][perms[c]] for c in range(N_CORES)]
    d_i = np.zeros(NBLK, np.int64)
    for c in range(N_CORES):
        dp = np.zeros(NPAD, np.int64)
        dp[:NPC] = deg_perm[c]
        d_i = np.maximum(d_i, dp.reshape(NBLK, 128).max(1))
    d_i = np.maximum(d_i, 1)

    # slot grid per core: for block i, slots[j, s] = src global padded row
    pad_idx = PAD_ROW - TBASE
    core_edges = []     # per core: (blk, j, slot, src_row) arrays
    for c in range(N_CORES):
        m = (dst >= c * NPC) & (dst < (c + 1) * NPC)
        s_c = src[m]
        dloc = inv_perms[c][dst[m] - c * NPC]     # permuted local pos
        core_edges.append((dloc, s_c))

    # uniform gather-unit split per block: units of up to 8 slot-cols
    units = []          # list of (blk, col0, ncols) — identical across cores
    for i in range(NBLK):
        s0 = 0
        while s0 < d_i[i]:
            nc_ = min(8, d_i[i] - s0)
            units.append((i, s0, int(nc_)))
            s0 += nc_
    idx_w = sum(u[2] * 8 for u in units)          # int16 cols in wrapped layout

    idx_arrs = []
    for c in range(N_CORES):
        dloc, s_c = core_edges[c]
        grid = np.full((NBLK, 128, int(d_i.max())), pad_idx, np.int64)
        order = np.argsort(dloc, kind="stable")
        dloc_s, src_s = dloc[order], s_c[order]
        # slot position within each node's list
        slot = np.zeros(len(dloc_s), np.int64)
        if len(dloc_s):
            new_node = np.r_[True, dloc_s[1:] != dloc_s[:-1]]
            idx0 = np.flatnonzero(new_node)
            counts = np.diff(np.r_[idx0, len(dloc_s)])
            slot = np.arange(len(dloc_s)) - np.repeat(idx0, counts)
        blk = dloc_s // 128
        j = dloc_s % 128
        grid[blk, j, slot] = row_of[src_s] - TBASE
        assert grid.min() >= -32768 and grid.max() <= 32767

        # tail-strip safety: last idx of every gather unit must be >= 0.
        for (i, s0, ncl) in units:
            last_col = s0 + ncl - 1
            if grid[i, 127, last_col] < 0:
                # swap within node (i, j=127)'s slot list
                row = grid[i, 127, :d_i[i]]
                cand = np.flatnonzero(row >= 0)
                if len(cand):
                    k = cand[0]
                    row[last_col], row[k] = row[k], row[last_col]
                else:
                    # all-negative full row: swap node j=127 with a node
                    # in this block that has a non-negative entry
                    done = False
                    for j2 in range(127):
                        r2 = grid[i, j2, :d_i[i]]
                        if (r2 >= 0).any() and r2[last_col] >= 0:
                            tmp = grid[i, 127].copy()
                            grid[i, 127] = grid[i, j2]
                            grid[i, j2] = tmp
                            # also swap the node assignment!
                            p = perms[c][i * 128 + 127]
                            q = perms[c][i * 128 + j2]
                            raise RuntimeError(
                                "node swap needed - unhandled; rerun with "
                                "different seed or implement")
                    if not done:
                        raise RuntimeError("cannot fix tail-strip; "
                                           "all-negative node row")
        # wrapped int16 layout per unit: flat i -> [i%16, i//16], 8x replicated
        parts = []
        for (i, s0, ncl) in units:
            flat = grid[i, :, s0:s0 + ncl].T.reshape(-1)   # slot-major: s*128+j
            w = flat.reshape(-1, 16).T                     # [16, n/16]
            parts.append(np.tile(w, (8, 1)))
        idx_arrs.append(np.concatenate(parts, axis=1).astype(np.int16))

    # ---- graph one-hot tiles per core per block
    g1h = []
    g1ht = []
    cnt = np.bincount(batch, minlength=G).astype(np.float64)
    for c in range(N_CORES):
        bperm = batch[c * NPC:(c + 1) * NPC][perms[c]]
        gm = np.zeros((NPAD, G), np.float32)
        gm[np.arange(NPC), bperm] = 1.0
        gmb = gm.reshape(NBLK, 128, G)
        # g1h: [128 j, NBLK*G] — block i's one-hot at cols [i*G, (i+1)*G)
        g1h.append(np.ascontiguousarray(
            gmb.transpose(1, 0, 2).reshape(128, NBLK * G)))
        # g1ht: [G, NBLK*128] — col (i*128 + j) = membership of node (i, j)
        g1ht.append(np.ascontiguousarray(gm.T))
    # x transposed + permuted + padded
    xT = []
    for c in range(N_CORES):
        xp = np.zeros((NPAD, x.shape[1]), np.float32)
        xp[:NPC] = x[c * NPC:(c + 1) * NPC][perms[c]]
        xT.append(np.ascontiguousarray(xp.T))

    # ---- weights
    in_W = np.asarray(inputs["in_W"], np.float32)
    in_b = np.asarray(inputs["in_b"], np.float32)
    Wg = np.asarray(inputs["Wg"], np.float32)
    att_src = np.asarray(inputs["att_src"], np.float32)
    att_dst = np.asarray(inputs["att_dst"], np.float32)
    gat_b = np.asarray(inputs["gat_b"], np.float32)
    gn_w = np.asarray(inputs["gn_w"], np.float32)
    gn_b = np.asarray(inputs["gn_b"], np.float32)
    gn_s = np.asarray(inputs["gn_s"], np.float32)

    W_z = np.zeros((L, HID, HID), np.float32)
    W_ed = np.zeros((L, HID, H), np.float32)
    Tinv_bd = np.zeros((L, HID, HID), np.float32)
    for l in range(L):
        for h in range(H):
            a = att_src[l, h]                          # [16]
            # T columns: [a | orthonormal complement of a]
            rng = np.random.default_rng(1234 + l * 16 + h)
            M = np.concatenate([a[:, None],
                                rng.standard_normal((DH, DH - 1))], 1)
            q, _ = np.linalg.qr(M)
            T = np.concatenate([a[:, None], q[:, 1:]], 1)  # [16,16]
            Ti = np.linalg.inv(T)
            sl = slice(h * DH, (h + 1) * DH)
            W_z[l][:, sl] = Wg[l][:, sl] @ T
            Tinv_bd[l][sl, sl] = Ti
            W_ed[l][:, h] = Wg[l][:, sl] @ att_dst[l, h]

    cnt_recip = np.zeros(G, np.float32)
    nz = cnt > 0
    cnt_recip[nz] = (1.0 / cnt[nz]).astype(np.float32)

    s = gn_s  # [L, HID]
    s2c = 2.0 * s - s * s                                  # (2s - s^2) per f

    consts = dict(
        inw=in_W,                                          # [F_in, 128]
        inb=in_b.reshape(HID, 1),                          # [128,1]
        wz=W_z, wed=W_ed, tinv=Tinv_bd,
        gatb=np.ascontiguousarray(gat_b.T),                # [128, L]
        gnw=np.ascontiguousarray(gn_w.T),                  # [128, L]
        gnb=np.ascontiguousarray(gn_b.T),
        gns=np.ascontiguousarray(s.T),
        gns2c=np.ascontiguousarray(s2c.T),
        cntr=np.tile(cnt_recip[None, :], (HID, 1)),        # [128, G]
        ident=np.eye(HID, dtype=np.float32),
        padrow=_padrow(),
    )

    return dict(d=d, units=units, d_i=d_i, idx_w=idx_w,
                idx_arrs=idx_arrs, g1h=g1h, g1ht=g1ht, xT=xT,
                perms=perms, inv_perms=inv_perms, consts=consts,
                batch=batch)


# ------------------------------------------------- numpy device emulation
# Mirrors the device program exactly (layouts, pads, bf16 rounding at the
# table) so host logic can be validated without a compile.


def _bf16(a):
    import ml_dtypes
    return a.astype(ml_dtypes.bfloat16).astype(np.float32)


def _numpy_pipeline(prep, dbg=None):
    d = prep["d"]
    L, G = d["L"], d["G"]
    NPC, NBLK, NPAD, TROWS, TBASE = (
        d["NPC"], d["NBLK"], d["NPAD"], d["TROWS"], d["TBASE"])
    C = prep["consts"]
    units, d_i = prep["units"], prep["d_i"]

    # input proj (per core, [128 f, NPAD n])
    hT = [C["inw"].T @ prep["xT"][c] + C["inb"] for c in range(N_CORES)]
    if dbg is not None:
        dbg["h0"] = [h.copy() for h in hT]

    for l in range(L):
        # ---- table build + allgather
        tbl = np.zeros((TROWS, HID), np.float32)
        eds = []
        for c in range(N_CORES):
            zT = C["wz"][l].T @ hT[c]                     # [128, NPAD]
            edT = C["wed"][l].T @ hT[c]                   # [H, NPAD]
            rows = _bf16(zT.T)                            # [NPAD, 128] bf16
            tbl[c * NPAD:(c + 1) * NPAD] = rows
            eds.append(edT)
        for c in range(N_CORES):
            tbl[(c + 1) * NPAD - 1] = 0.0
            tbl[(c + 1) * NPAD - 1, 0::DH] = PAD_ES
        tblv = tbl  # already bf16-rounded
        if dbg is not None and l == 0:
            dbg["tbl0"] = tbl.copy()
            dbg["ed0"] = [e.copy() for e in eds]

        # ---- edge phase per core
        new_hT = []
        stats = np.zeros((N_CORES, HID, G, 2), np.float32)
        for c in range(N_CORES):
            idx = prep["idx_arrs"][c]
            xt_new = np.zeros((HID, NPAD), np.float32)
            col = 0
            # rebuild grid from wrapped idx (validates wrapping too)
            for i in range(NBLK):
                di = int(d_i[i])
                msg = np.zeros((128, di, HID), np.float32)
                s0 = 0
                while s0 < di:
                    ncl = min(8, di - s0)
                    w = idx[:16, col:col + ncl * 8]        # [16, n/16]
                    flat = w.T.reshape(-1)                 # i -> idx
                    col += ncl * 8
                    rows = tblv[flat.astype(np.int64) + TBASE]
                    msg[:, s0:s0 + ncl, :] = (
                        rows.reshape(ncl, 128, HID).transpose(1, 0, 2))
                    s0 += ncl
                if dbg is not None and l == 0 and i == 0 and c == 0:
                    dbg["msg0"] = msg.copy()
                es = msg[:, :, 0::DH]                      # [128, di, H]
                ed = eds[c][:, i * 128:(i + 1) * 128].T    # [128, H]
                e = es + ed[:, None, :]
                if dbg is not None and l == 0 and i == 0 and c == 0:
                    dbg["e0"] = e.copy()
                e = np.where(e >= 0, e, NEG_SLOPE * e)
                ex = np.exp(e)                             # [128, di, H]
                if dbg is not None and l == 0 and i == 0 and c == 0:
                    dbg["ex0"] = ex.copy()
                denom = ex.sum(1)                          # [128, H]
                if dbg is not None and l == 0 and i == 0 and c == 0:
                    dbg["den0"] = denom.copy()
                exb = _bf16(ex)
                mp = _bf16(msg * exb.repeat(DH, axis=2))   # M' bf16 in-place
                num = mp.sum(1)                            # [128 j, 128 f]
                gat = num * (1.0 / denom).repeat(DH, axis=1)
                if dbg is not None and l == 0 and i == 0 and c == 0:
                    dbg["num0"] = num.copy(); dbg["gat0"] = gat.copy()
                attnT = C["tinv"][l].T @ gat.T             # [f', j]
                xt = attnT + hT[c][:, i * 128:(i + 1) * 128] + C["gatb"][:, l:l + 1]
                xt_new[:, i * 128:(i + 1) * 128] = xt
                # stats: contraction over j with the block's one-hot
                g1hb = prep["g1h"][c][:, i * G:(i + 1) * G]  # [128 j, G]
                stats[c, :, :, 0] += xt @ g1hb               # sum x: [f, G]
                stats[c, :, :, 1] += (xt * xt) @ g1hb
            new_hT.append(xt_new)

        # ---- allreduce stats + norm
        tot = stats.sum(0)                                 # [f, G, 2]
        if dbg is not None and l == 0:
            dbg["st0"] = tot.copy()
            dbg["x0"] = [h.copy() for h in new_hT]
        mean = tot[:, :, 0] * C["cntr"]
        ex2 = tot[:, :, 1] * C["cntr"]
        var = ex2 - C["gns2c"][:, l:l + 1] * mean * mean
        rstd = 1.0 / np.sqrt(var + EPS)
        c1 = C["gnw"][:, l:l + 1] * rstd                   # [f, G]
        c0 = C["gnb"][:, l:l + 1] - C["gns"][:, l:l + 1] * mean * c1
        for c in range(N_CORES):
            g1ht = prep["g1ht"][c]                         # [G, NBLK*128]
            p1 = c1 @ g1ht                                 # [f, NPAD]
            p0 = c0 @ g1ht
            hT[c] = new_hT[c] * p1 + p0

    # ---- output assembly: hT[c][:, p] holds node perms[c][p]
    N = d["N"]
    out = np.zeros((N, HID), np.float32)
    for c in range(N_CORES):
        out[c * NPC + prep["perms"][c]] = hT[c][:, :NPC].T
    return out


# ---------------------------------------------------------------- device


def _build_program(prep, timing_reps=None, dbg=True, ablate=()):
    import contextlib

    import concourse.tile as tile
    from concourse import bacc, mybir, library_config

    d = prep["d"]
    L, G = d["L"], d["G"]
    NPC, NBLK, NPAD, TROWS, TBASE = (
        d["NPC"], d["NBLK"], d["NPAD"], d["TROWS"], d["TBASE"])
    units, d_i, idx_w = prep["units"], prep["d_i"], prep["idx_w"]
    DIMAX = int(d_i.max())
    F_IN = prep["xT"][0].shape[0]
    NCHUNK = (NPAD + 511) // 512
    AF = mybir.ActivationFunctionType
    ALU = mybir.AluOpType

    f32, bf16, i16 = mybir.dt.float32, mybir.dt.bfloat16, mybir.dt.int16

    nc = bacc.Bacc(None, target_bir_lowering=False, num_swdge_queues=4)

    def param(name, shape, dtype=f32, out=False):
        return nc.declare_dram_parameter(name, list(shape), dtype, isOutput=out)

    P = dict(
        xT=param("xT", [F_IN, NPAD]),
        idx=param("idx", [128, idx_w], i16),
        g1h=param("g1h", [128, NBLK * G]),
        g1ht=param("g1ht", [G, NBLK * 128]),
        inw=param("inw", [F_IN, HID]),
        inb=param("inb", [HID, 1]),
        wz=param("wz", [L, HID, HID]),
        wed=param("wed", [L, HID, H]),
        tinv=param("tinv", [L, HID, HID]),
        gatb=param("gatb", [HID, L]),
        gnw=param("gnw", [HID, L]),
        gnb=param("gnb", [HID, L]),
        gns=param("gns", [HID, L]),
        gns2c=param("gns2c", [HID, L]),
        cntr=param("cntr", [HID, G]),
        ident=param("ident", [HID, HID]),
        padrow=param("padrow", [1, HID], bf16),
        out=param("out", [NPC, HID], out=True),
        dbg_h0=param("dbg_h0", [HID, NPAD], out=True),
        dbg_x0=param("dbg_x0", [HID, NPAD], out=True),
        dbg_tbl=param("dbg_tbl", [TROWS, HID], bf16, out=True),
        dbg_ed=param("dbg_ed", [128, NBLK * H], out=True),
        dbg_st=param("dbg_st", [HID, 2 * G], out=True),
        dbg_edt=param("dbg_edt", [H, NPAD], out=True),
    )

    qctr = [0]

    def next_q():
        q = qctr[0] % 4
        qctr[0] += 1
        return q

    with tile.TileContext(nc) as tc:
        est = contextlib.ExitStack()
        singles = est.enter_context(tc.tile_pool(name="singles", bufs=1))
        msgpA = est.enter_context(tc.tile_pool(name="msgA", bufs=3))
        msgpB = est.enter_context(tc.tile_pool(name="msgB", bufs=8))
        ep = est.enter_context(tc.tile_pool(name="etile", bufs=7))
        blkp = est.enter_context(tc.tile_pool(name="blk", bufs=5))
        stag = est.enter_context(tc.tile_pool(name="stag", bufs=3))
        psA = est.enter_context(tc.tile_pool(name="psA", bufs=1, space="PSUM"))
        psnp = est.enter_context(tc.tile_pool(name="psnp", bufs=2, space="PSUM"))
        pssm = est.enter_context(tc.tile_pool(name="pssm", bufs=3, space="PSUM"))
        psacc = est.enter_context(tc.tile_pool(name="psacc", bufs=1, space="PSUM"))
        dram = est.enter_context(tc.tile_pool(name="dram", bufs=1, space="DRAM"))

        nc.gpsimd.load_library(library_config.mlp)

        # ---------------- constants
        def load(t, src):
            nc.sync.dma_start(out=t, in_=src)
            return t

        ident_f = load(singles.tile([HID, HID], f32, name="idf"), P["ident"][:])
        ident_b = singles.tile([HID, HID], bf16, name="idb")
        nc.vector.tensor_copy(out=ident_b, in_=ident_f)
        idx_sb = load(singles.tile([128, idx_w], i16, name="idxs"), P["idx"][:])
        g1h_sb = load(singles.tile([128, NBLK * G], f32, name="g1h"), P["g1h"][:])
        inw_sb = load(singles.tile([F_IN, HID], f32, name="inw"), P["inw"][:])
        inb_sb = load(singles.tile([HID, 1], f32, name="inb"), P["inb"][:])
        wz_sb = [load(singles.tile([HID, HID], f32, name=f"wz{l}"), P["wz"][l])
                 for l in range(L)]
        wed_sb = [load(singles.tile([HID, H], f32, name=f"wed{l}"), P["wed"][l])
                  for l in range(L)]
        tinv_sb = [load(singles.tile([HID, HID], f32, name=f"ti{l}"), P["tinv"][l])
                   for l in range(L)]
        gatb_sb = load(singles.tile([HID, L], f32, name="gatb"), P["gatb"][:])
        gnw_sb = load(singles.tile([HID, L], f32, name="gnw"), P["gnw"][:])
        gnb_sb = load(singles.tile([HID, L], f32, name="gnb"), P["gnb"][:])
        gns_sb = load(singles.tile([HID, L], f32, name="gns"), P["gns"][:])
        gns2c_sb = load(singles.tile([HID, L], f32, name="gns2c"), P["gns2c"][:])
        cntr_sb = load(singles.tile([HID, G], f32, name="cntr"), P["cntr"][:])
        eps_sb = singles.tile([HID, 1], f32, name="eps")
        nc.vector.memset(eps_sb, EPS)

        h_a = singles.tile([HID, NPAD], f32, name="h_a")
        ed_all = singles.tile([128, NBLK * H], f32, name="ed_all")

        tbl_in = [dram.tile([NPAD, HID], bf16, name=f"tin{l}") for l in range(L)]
        tbl = [dram.tile([TROWS, HID], bf16, addr_space="Shared", name=f"tbl{l}")
               for l in range(L)]
        st_in = [dram.tile([HID, 2 * G], f32, name=f"st_in{l}")
                 for l in range(L)]
        st_out = [dram.tile([HID, 2 * G], f32, addr_space="Shared",
                            name=f"st_out{l}") for l in range(L)]

        # ---------------- input projection: h0^T = inw^T @ x^T + b
        for k in range(NCHUNK):
            c0, c1_ = k * 512, min((k + 1) * 512, NPAD)
            w = c1_ - c0
            xt = stag.tile([F_IN, 512], f32, name="xchunk")
            nc.sync.dma_start(out=xt[:, :w], in_=P["xT"][:, c0:c1_])
            ps = psA.tile([HID, 512], f32, name="psbig")
            nc.tensor.matmul(out=ps[:, :w], lhsT=inw_sb, rhs=xt[:, :w],
                             start=True, stop=True)
            nc.scalar.activation(out=h_a[:, c0:c1_], in_=ps[:, :w],
                                 func=AF.Identity, bias=inb_sb[:, 0:1])

        # ---------------- gather table build for layer l from h_src
        def build_table(l, h_src):
            for k in range(NCHUNK):
                c0, c1_ = k * 512, min((k + 1) * 512, NPAD)
                w = c1_ - c0
                psz = psA.tile([HID, 512], f32, name="psbig")
                nc.tensor.matmul(out=psz[:, :w], lhsT=wz_sb[l],
                                 rhs=h_src[:, c0:c1_], start=True, stop=True)
                zt = stag.tile([HID, 512], f32, name="zt")
                nc.scalar.activation(out=zt[:, :w], in_=psz[:, :w], func=AF.Copy)
                pse = psA.tile([HID, 512], f32, name="psbig")
                nc.tensor.matmul(out=pse[:H, :w], lhsT=wed_sb[l],
                                 rhs=h_src[:, c0:c1_], start=True, stop=True)
                edt = stag.tile([H, 512], f32, name="edt")
                nc.vector.tensor_copy(out=edt[:, :w], in_=pse[:H, :w])
                if l == 0 and dbg:
                    nc.sync.dma_start(out=P["dbg_edt"][:, c0:c1_],
                                      in_=edt[:, :w])
                for bb in range(w // 128):
                    i = (c0 + bb * 128) // 128
                    pst = pssm.tile([128, HID], f32, name="pssm")
                    nc.tensor.matmul(
                        out=pst, lhsT=zt[:, bb * 128:(bb + 1) * 128],
                        rhs=ident_f, start=True, stop=True)
                    rows = stag.tile([128, HID], bf16, name="rows")
                    nc.scalar.activation(out=rows, in_=pst, func=AF.Copy)
                    nc.sync.dma_start(out=tbl_in[l][i * 128:(i + 1) * 128, :],
                                      in_=rows)
                    if i == NBLK - 1:
                        nc.sync.dma_start(
                            out=tbl_in[l][NPAD - 1:NPAD, :],
                            in_=P["padrow"][:])
                    psd = pssm.tile([128, HID], f32, name="pssm")
                    nc.tensor.matmul(
                        out=psd[:, :H], lhsT=edt[:, bb * 128:(bb + 1) * 128],
                        rhs=ident_f[:H, :H], start=True, stop=True)
                    nc.vector.tensor_copy(out=ed_all[:, i * H:(i + 1) * H],
                                          in_=psd[:, :H])
            if timing_reps is None:
                nc.gpsimd.collective_compute(
                    "AllGather", mybir.AluOpType.bypass,
                    replica_groups=[list(range(N_CORES))],
                    ins=[tbl_in[l].opt()], outs=[tbl[l].opt()])
            else:
                nc.sync.dma_start(out=tbl[l][:NPAD, :], in_=tbl_in[l][:])

        # ---------------- edge phase: h_src -> h_dst (pre-norm x^T)
        # 3-stage software pipeline across blocks so each engine's stream
        # interleaves independent blocks (in-order engines stall otherwise).
        stats1 = psacc.tile([HID, G], f32, name="sa")
        stats2 = psacc.tile([HID, G], f32, name="sb")

        def edge_phase(l, h_src, h_dst):
            ucol = {}
            col = 0
            for (i, s0, ncl) in units:
                ucol[(i, s0)] = col
                col += ncl * 8
            DMID = min(20, DIMAX)
            state = {}

            def stage1(i):  # gathers + attention scalars
                di = int(d_i[i])
                if di > DMID:
                    msg = msgpA.tile([128, DIMAX, HID], bf16, name="msgA")
                else:
                    msg = msgpB.tile([128, DMID, HID], bf16, name="msgB")
                s0 = 0
                while s0 < di:
                    ncl = min(8, di - s0)
                    c0 = ucol[(i, s0)]
                    nidx = ncl * 128
                    if "gather" not in ablate:
                        nc.gpsimd.dma_gather(
                            out_ap=msg[:, s0:s0 + ncl, :],
                            in_ap=tbl[l][TBASE:, :],
                            idxs_ap=idx_sb[:, c0:c0 + ncl * 8],
                            num_idxs=nidx, num_idxs_reg=nidx, elem_size=HID,
                            queue_num=next_q())
                    else:
                        nc.vector.memset(msg[:, s0:s0 + ncl, :], 0.25)
                    s0 += ncl
                e_t = ep.tile([128, DIMAX, H], f32, name="e_t")
                nc.vector.tensor_tensor(
                    out=e_t[:, :di, :],
                    in0=msg[:, :di, 0:HID:DH],
                    in1=ed_all[:, i * H:(i + 1) * H].unsqueeze(1)
                        .to_broadcast([128, di, H]),
                    op=ALU.add)
                # leaky_relu(x) = (1+a)/2*x + (1-a)/2*|x|
                ab_t = ep.tile([128, DIMAX, H], f32, name="ab_t")
                nc.scalar.activation(out=ab_t[:, :di, :], in_=e_t[:, :di, :],
                                     func=AF.Abs,
                                     scale=(1.0 - NEG_SLOPE) / 2.0)
                nc.vector.tensor_scalar_mul(
                    e_t[:, :di, :], e_t[:, :di, :], (1.0 + NEG_SLOPE) / 2.0)
                nc.vector.tensor_tensor(out=e_t[:, :di, :], in0=e_t[:, :di, :],
                                        in1=ab_t[:, :di, :], op=ALU.add)
                exb = ep.tile([128, DIMAX, H], bf16, name="exb")
                nc.scalar.activation(out=exb[:, :di, :], in_=e_t[:, :di, :],
                                     func=AF.Exp)
                den = blkp.tile([128, H], f32, name="den")
                nc.vector.tensor_reduce(
                    out=den, in_=exb[:, :di, :].rearrange("p a b -> p b a"),
                    axis=mybir.AxisListType.X, op=ALU.add)
                state[i] = (msg, exb, den)

            def stage2(i):  # weight + aggregate + alpha-normalize
                di = int(d_i[i])
                msg, exb, den = state[i]
                nc.vector.tensor_tensor(
                    out=msg[:, :di, :].rearrange("p a (b c) -> p a b c", b=H),
                    in0=msg[:, :di, :].rearrange("p a (b c) -> p a b c", b=H),
                    in1=exb[:, :di, :].unsqueeze(3)
                        .to_broadcast([128, di, H, DH]),
                    op=ALU.mult)
                nps = psnp.tile([128, HID], f32, name="psnp")
                for sj in range(di):
                    nc.tensor.matmul(out=nps, lhsT=ident_b, rhs=msg[:, sj, :],
                                     start=(sj == 0), stop=(sj == di - 1))
                rec = blkp.tile([128, H], f32, name="rec")
                nc.vector.reciprocal(out=rec, in_=den)
                gat = blkp.tile([128, HID], f32, name="gat")
                nc.vector.tensor_tensor(
                    out=gat.rearrange("p (a b) -> p a b", a=H),
                    in0=nps.rearrange("p (a b) -> p a b", a=H),
                    in1=rec.unsqueeze(2).to_broadcast([128, H, DH]),
                    op=ALU.mult)
                state[i] = gat

            def stage3(i):  # unrotate + residual + stats
                gat = state.pop(i)
                pgt = pssm.tile([128, HID], f32, name="pssm")
                nc.tensor.matmul(out=pgt, lhsT=gat, rhs=ident_f,
                                 start=True, stop=True)
                gt = blkp.tile([128, HID], f32, name="gt")
                nc.vector.tensor_copy(out=gt, in_=pgt)
                pat = pssm.tile([128, HID], f32, name="pssm")
                nc.tensor.matmul(out=pat, lhsT=tinv_sb[l], rhs=gt,
                                 start=True, stop=True)
                xt = blkp.tile([128, HID], f32, name="xt")
                nc.scalar.activation(out=xt, in_=pat, func=AF.Identity,
                                     bias=gatb_sb[:, l:l + 1])
                sl = slice(i * 128, (i + 1) * 128)
                nc.vector.tensor_tensor(out=h_dst[:, sl], in0=xt,
                                        in1=h_src[:, sl], op=ALU.add)
                if "stats" in ablate:
                    return
                pxb = pssm.tile([128, HID], f32, name="pssm")
                nc.tensor.matmul(out=pxb, lhsT=h_dst[:, sl], rhs=ident_f,
                                 start=True, stop=True)
                xb = blkp.tile([128, HID], f32, name="xb")
                nc.vector.tensor_copy(out=xb, in_=pxb)
                sq = blkp.tile([128, HID], f32, name="sq")
                nc.scalar.activation(out=sq, in_=xb, func=AF.Square)
                nc.tensor.matmul(out=stats1, lhsT=xb,
                                 rhs=g1h_sb[:, i * G:(i + 1) * G],
                                 start=(i == 0), stop=(i == NBLK - 1),
                                 skip_group_check=True)
                nc.tensor.matmul(out=stats2, lhsT=sq,
                                 rhs=g1h_sb[:, i * G:(i + 1) * G],
                                 start=(i == 0), stop=(i == NBLK - 1),
                                 skip_group_check=True)

            for i in range(NBLK + 2):
                if i < NBLK:
                    stage1(i)
                if 1 <= i <= NBLK and "post" not in ablate:
                    stage2(i - 1)
                if 2 <= i <= NBLK + 1 and "post" not in ablate:
                    stage3(i - 2)
                if "post" in ablate and i < NBLK:
                    _, _, den = state.pop(i)
                    nc.vector.tensor_copy(
                        out=h_dst[:, i * 128 + 0:i * 128 + 8],
                        in_=den)
            stl = stag.tile([HID, 2 * G], f32, name="stl")
            if "post" in ablate or "stats" in ablate:
                nc.vector.memset(stl, 1.0)
            else:
                nc.vector.tensor_copy(out=stl[:, :G], in_=stats1)
                nc.vector.tensor_copy(out=stl[:, G:], in_=stats2)
            nc.sync.dma_start(out=st_in[l], in_=stl)
            if timing_reps is None:
                nc.gpsimd.collective_compute(
                    "AllReduce", mybir.AluOpType.add,
                    replica_groups=[list(range(N_CORES))],
                    ins=[st_in[l].opt()], outs=[st_out[l].opt()])
            else:
                nc.sync.dma_start(out=st_out[l][:], in_=st_in[l][:])
            stg = stag.tile([HID, 2 * G], f32, name="stg")
            nc.sync.dma_start(out=stg, in_=st_out[l])
            return stg

        # ---------------- graph norm applied to h (in place)
        def norm_apply(l, stg, h):
            mean = blkp.tile([128, G], f32, name="mean")
            nc.vector.tensor_tensor(out=mean, in0=stg[:, :G], in1=cntr_sb,
                                    op=ALU.mult)
            ex2 = blkp.tile([128, G], f32, name="ex2")
            nc.vector.tensor_tensor(out=ex2, in0=stg[:, G:], in1=cntr_sb,
                                    op=ALU.mult)
            m2 = blkp.tile([128, G], f32, name="m2")
            nc.vector.tensor_tensor(out=m2, in0=mean, in1=mean, op=ALU.mult)
            nc.vector.tensor_tensor(
                out=m2, in0=m2,
                in1=gns2c_sb[:, l:l + 1].to_broadcast([HID, G]),
                op=ALU.mult)
            var = blkp.tile([128, G], f32, name="var")
            nc.vector.tensor_tensor(out=var, in0=ex2, in1=m2, op=ALU.subtract)
            nc.scalar.activation(out=var, in_=var, func=AF.Sqrt, bias=eps_sb[:, 0:1])
            rstd = blkp.tile([128, G], f32, name="rstd")
            nc.vector.reciprocal(out=rstd, in_=var)
            c1 = blkp.tile([128, G], f32, name="c1")
            nc.vector.tensor_tensor(
                out=c1, in0=rstd,
                in1=gnw_sb[:, l:l + 1].to_broadcast([HID, G]), op=ALU.mult)
            c0t = blkp.tile([128, G], f32, name="c0t")
            nc.vector.tensor_tensor(out=c0t, in0=mean, in1=c1, op=ALU.mult)
            nc.vector.tensor_tensor(
                out=c0t, in0=c0t,
                in1=gns_sb[:, l:l + 1].to_broadcast([HID, G]), op=ALU.mult)
            c0 = blkp.tile([128, G], f32, name="c0")
            nc.vector.tensor_tensor(
                out=c0, in0=gnb_sb[:, l:l + 1].to_broadcast([HID, G]),
                in1=c0t, op=ALU.subtract)
            pc = pssm.tile([128, HID], f32, name="pssm")
            nc.tensor.matmul(out=pc[:G, :], lhsT=c1, rhs=ident_f,
                             start=True, stop=True)
            c1T = blkp.tile([G, HID], f32, name="c1T")
            nc.vector.tensor_copy(out=c1T, in_=pc[:G, :])
            pc2 = pssm.tile([128, HID], f32, name="pssm")
            nc.tensor.matmul(out=pc2[:G, :], lhsT=c0, rhs=ident_f,
                             start=True, stop=True)
            c0T = blkp.tile([G, HID], f32, name="c0T")
            nc.vector.tensor_copy(out=c0T, in_=pc2[:G, :])
            for i in range(NBLK):
                g1htb = stag.tile([G, 128], f32, name="g1htb")
                nc.sync.dma_start(out=g1htb,
                                  in_=P["g1ht"][:, i * 128:(i + 1) * 128])
                p1 = pssm.tile([128, HID], f32, name="pssm")
                nc.tensor.matmul(out=p1, lhsT=c1T, rhs=g1htb,
                                 start=True, stop=True)
                p0 = pssm.tile([128, HID], f32, name="pssm")
                nc.tensor.matmul(out=p0, lhsT=c0T, rhs=g1htb,
                                 start=True, stop=True)
                sl = slice(i * 128, (i + 1) * 128)
                nc.vector.tensor_tensor(out=h[:, sl], in0=h[:, sl], in1=p1,
                                        op=ALU.mult)
                nc.vector.tensor_tensor(out=h[:, sl], in0=h[:, sl], in1=p0,
                                        op=ALU.add)

        # ---------------- layers
        h_src = h_dst = h_a
        loop_ctx = tc.For_i(0, timing_reps) if timing_reps else None
        if loop_ctx:
            loop_ctx.__enter__()
        for l in range(L):
            build_table(l, h_src)
            if l == 0 and dbg:
                nc.sync.dma_start(out=P["dbg_h0"][:], in_=h_src)
                nc.sync.dma_start(out=P["dbg_tbl"][:], in_=tbl[0][:])
                nc.sync.dma_start(out=P["dbg_ed"][:], in_=ed_all)
            stg = edge_phase(l, h_src, h_dst)
            if l == 0 and dbg:
                nc.sync.dma_start(out=P["dbg_st"][:], in_=stg)
                nc.sync.dma_start(out=P["dbg_x0"][:], in_=h_dst)
            norm_apply(l, stg, h_dst)
        if loop_ctx:
            loop_ctx.__exit__(None, None, None)

        # ---------------- output rows
        for i in range(NBLK):
            r0 = i * 128
            r1 = min(r0 + 128, NPC)
            if r1 <= r0:
                break
            po = pssm.tile([128, HID], f32, name="pssm")
            nc.tensor.matmul(out=po, lhsT=h_src[:, r0:r0 + 128], rhs=ident_f,
                             start=True, stop=True)
            rows = stag.tile([128, HID], f32, name="orow")
            nc.vector.tensor_copy(out=rows, in_=po)
            nc.sync.dma_start(out=P["out"][r0:r1, :], in_=rows[:r1 - r0, :])

        est.close()

    nc.compile()
    return nc


def _make_inmaps(prep):
    import ml_dtypes
    C = prep["consts"]
    maps = []
    for c in range(N_CORES):
        m = dict(
            xT=prep["xT"][c],
            idx=prep["idx_arrs"][c],
            g1h=prep["g1h"][c],
            g1ht=prep["g1ht"][c],
            inw=C["inw"], inb=C["inb"], wz=C["wz"], wed=C["wed"],
            tinv=C["tinv"], gatb=C["gatb"], gnw=C["gnw"], gnb=C["gnb"],
            gns=C["gns"], gns2c=C["gns2c"], cntr=C["cntr"], ident=C["ident"],
            padrow=C["padrow"],
        )
        maps.append(m)
    return maps


def _assemble(prep, results):
    d = prep["d"]
    NPC = d["NPC"]
    out = np.zeros((d["N"], HID), np.float32)
    for c in range(N_CORES):
        out[c * NPC + prep["perms"][c]] = results[c]["out"]
    return out


def _run(inputs, cfg):
    from concourse.bass_utils import run_bass_kernel_spmd
    prep = _preprocess(inputs, cfg)
    nc = _build_program(prep)
    res = run_bass_kernel_spmd(nc, _make_inmaps(prep),
                               core_ids=list(range(N_CORES)))
    return _assemble(prep, res.results)


def kernel(**inputs):
    return _run(inputs, FULL_CFG)

